# revision 2
# baseline (speedup 1.0000x reference)
"""Trainium2 Bass kernel for nn_Attention_33354716021131.

Dense GQA attention block (B=2, S=2048, D=4096, 32 q-heads / 8 kv-heads,
head_dim 128, RoPE, causal softmax) tensor-parallel across 8 NeuronCores.

Sharding (per core c):
  - heads: q-heads 4c..4c+3 (one kv-head group c) -> wq/wk/wv column shards
  - x transposed cooperatively: core c transposes x[:, 512c:512c+512] on the
    PE, AllGather -> full x^T on every core
  - attention entirely local to the core (its 4 q-heads x 2 batches)
  - attention outputs (head-major, transposed) AllGather -> full O^T, then
    wo column shard: core c computes y[:, 512c:512c+512]; host concatenates.

Everything stays in "transposed" [feature, token] layout between the input
transpose and the final wo projection, which makes every matmul a clean
[K=128 partition] x [N=512 free] fp32r instruction:
  - QKV:   qT/kT/vT tile = w_tile.T @ xT_tile            (accumulate over k)
  - RoPE:  pair-swap via a constant permutation matmul, cos/sin via DVE
  - S^T:   sT[k,q] = kT_tile.T @ qT_tile   (keys on partitions)
  - P^T:   exp on ScalarE (scale fused), causal tri-mask on diagonal tiles
  - PV:    oT[d,q] += v_nat_tile.T @ pT_tile; denominators via ones-matmul
  - WO:    y[tok, cols] = oT_tile.T @ wo_tile
All matmul operands are float32r (fp32 bits, PE reduced-precision mode,
4x faster than fp32; measured ~1.5e-4 rel err at K=4096).
"""
import math
import os

import numpy as np

N_CORES = 8
B = 2
S = 2048
DM = 4096
N_HEADS = 32
HD = 128
NQH = N_HEADS // N_CORES          # 4 q heads per core
HDQ = NQH * HD                    # 512
T = B * S                         # 4096 tokens
KC = DM // 128                    # 32 k-chunks
TB = 512                          # token block for projections
NTB = S // TB                     # 4 per batch
QB = 512                          # query block for attention
NQB = S // QB                     # 4
NKT = S // 128                    # 16 key tiles per batch
SCALE = 1.0 / math.sqrt(HD)
ROPE_THETA = 10000.0

_CACHE = {}


def _consts():
    i = np.arange(HD // 2)
    inv = 1.0 / (ROPE_THETA ** (2 * i / HD))
    t = np.arange(S)
    ang = np.outer(inv, t)  # [64, S]
    cosT = np.repeat(np.cos(ang), 2, axis=0).astype(np.float32)  # [128, S]
    sinT = np.repeat(np.sin(ang), 2, axis=0).astype(np.float32)
    perm = np.zeros((128, 128), np.float32)
    for j in range(64):
        perm[2 * j, 2 * j + 1] = 1.0
        perm[2 * j + 1, 2 * j] = -1.0
    tri = (np.arange(128)[:, None] <= np.arange(128)[None, :]).astype(np.float32)
    ident = np.eye(128, dtype=np.float32)
    ones = np.ones((128, 1), np.float32)
    return cosT, sinT, perm, tri, ident, ones


def _build():
    import concourse.mybir as mybir
    import concourse.tile as tile
    from concourse import bacc

    F32 = mybir.dt.float32
    F32R = mybir.dt.float32r

    nc = bacc.Bacc("TRN2", target_bir_lowering=False, debug=False,
                   num_devices=N_CORES)

    xs = nc.dram_tensor("xs", [T, HDQ], F32, kind="ExternalInput")      # x[:, dim slice]
    wq = nc.dram_tensor("wq", [DM, HDQ], F32, kind="ExternalInput")
    wk = nc.dram_tensor("wk", [DM, HD], F32, kind="ExternalInput")
    wv = nc.dram_tensor("wv", [DM, HD], F32, kind="ExternalInput")
    wo = nc.dram_tensor("wo", [DM, HDQ], F32, kind="ExternalInput")     # wo[:, col slice]
    cosc = nc.dram_tensor("cosc", [128, S], F32, kind="ExternalInput")
    sinc = nc.dram_tensor("sinc", [128, S], F32, kind="ExternalInput")
    permc = nc.dram_tensor("permc", [128, 128], F32, kind="ExternalInput")
    tric = nc.dram_tensor("tric", [128, 128], F32, kind="ExternalInput")
    identc = nc.dram_tensor("identc", [128, 128], F32, kind="ExternalInput")
    onesc = nc.dram_tensor("onesc", [128, 1], F32, kind="ExternalInput")

    y = nc.dram_tensor("y", [T, HDQ], F32, kind="ExternalOutput")       # y[:, col slice]

    rg = [list(range(N_CORES))]

    with tile.TileContext(nc) as tc:
        with (
            tc.tile_pool(name="dram", bufs=1, space="DRAM") as dram,
            tc.tile_pool(name="const", bufs=1) as cp,
        ):
            # DRAM bounce buffers (xT / oT split by batch so the AllGathers
            # pipeline against compute)
            xT_h = [dram.tile([HDQ, S], F32R, name=f"xT_h{b}") for b in range(B)]
            xT_F = [dram.tile([DM, S], F32R, addr_space="Shared", name=f"xT_F{b}")
                    for b in range(B)]
            oT_h = [dram.tile([HDQ, S], F32R, name=f"oT_h{b}") for b in range(B)]
            oT_F = [dram.tile([DM, S], F32R, addr_space="Shared", name=f"oT_F{b}")
                    for b in range(B)]

            cos_sb = cp.tile([128, S], F32, tag="cos")
            sin_sb = cp.tile([128, S], F32, tag="sin")
            perm_sb = cp.tile([128, 128], F32R, tag="perm")
            tri_sb = cp.tile([128, 128], F32, tag="tri")
            id_sb = cp.tile([128, 128], F32, tag="id")
            ones_sb = cp.tile([128, 1], F32R, tag="ones")
            nc.sync.dma_start(out=cos_sb[:], in_=cosc.ap())
            nc.sync.dma_start(out=sin_sb[:], in_=sinc.ap())
            nc.sync.dma_start(out=perm_sb[:], in_=permc.ap().bitcast(F32R))
            nc.sync.dma_start(out=tri_sb[:], in_=tric.ap())
            nc.sync.dma_start(out=id_sb[:], in_=identc.ap())
            nc.sync.dma_start(out=ones_sb[:], in_=onesc.ap().bitcast(F32R))

            # ---------- phase 0: transpose own x dim-slice, AllGather x^T ----
            with (
                tc.tile_pool(name="ps0", bufs=2, space="PSUM") as ps0,
                tc.tile_pool(name="w0", bufs=3) as wp,
            ):
                for b in range(B):
                    for dt_i in range(HDQ // 128):
                        for tt in range(S // 128):
                            row = b * S + tt * 128
                            x_tile = wp.tile([128, 128], F32, tag="x_tile")
                            nc.sync.dma_start(
                                out=x_tile[:],
                                in_=xs.ap()[row:row + 128,
                                            dt_i * 128:(dt_i + 1) * 128],
                            )
                            pt = ps0.tile([128, 128], F32, tag="pt")
                            nc.tensor.transpose(pt[:], x_tile[:], id_sb[:])
                            xt_sb = wp.tile([128, 128], F32R, tag="xt_sb")
                            nc.scalar.copy(xt_sb[:], pt[:])
                            nc.sync.dma_start(
                                out=xT_h[b][:][dt_i * 128:(dt_i + 1) * 128,
                                               tt * 128:(tt + 1) * 128],
                                in_=xt_sb[:],
                            )
                    nc.gpsimd.collective_compute(
                        "AllGather", mybir.AluOpType.bypass, replica_groups=rg,
                        ins=[xT_h[b][:].opt()], outs=[xT_F[b][:].opt()],
                    )

            # ---------- weights ----------
            wpool_cm = tc.tile_pool(name="wqkv", bufs=1)
            wpool = wpool_cm.__enter__()
            wq_sb = wpool.tile([128, KC * HDQ], F32R, tag="wq")
            wk_sb = wpool.tile([128, KC * HD], F32R, tag="wk")
            wv_sb = wpool.tile([128, KC * HD], F32R, tag="wv")
            nc.sync.dma_start(
                out=wq_sb[:].rearrange("p (kc d) -> p kc d", kc=KC),
                in_=wq.ap().rearrange("(kc p) d -> p kc d", p=128).bitcast(F32R),
            )
            nc.sync.dma_start(
                out=wk_sb[:].rearrange("p (kc d) -> p kc d", kc=KC),
                in_=wk.ap().rearrange("(kc p) d -> p kc d", p=128).bitcast(F32R),
            )
            nc.sync.dma_start(
                out=wv_sb[:].rearrange("p (kc d) -> p kc d", kc=KC),
                in_=wv.ap().rearrange("(kc p) d -> p kc d", p=128).bitcast(F32R),
            )

            # per-batch resident tiles (reused slot across the two batches)
            bp_cm = tc.tile_pool(name="batch", bufs=1)
            bp = bp_cm.__enter__()
            qT = [bp.tile([128, S], F32R, tag=f"qT{h}", name=f"qT{h}")
                  for h in range(NQH)]
            kT = bp.tile([128, S], F32R, tag="kT")
            v_nat = bp.tile([128, NKT * 128], F32R, tag="v_nat")

            for b in range(B):
                # ---------- QKV projection (transposed form) ----------
                with (
                    tc.tile_pool(name=f"ps_acc{b}", bufs=1, space="PSUM") as ps_acc,
                    tc.tile_pool(name=f"ps_rope{b}", bufs=1, space="PSUM") as ps_rope,
                    tc.tile_pool(name=f"wq{b}", bufs=2) as wp,
                ):
                    for tb in range(NTB):
                        tsl = slice(tb * TB, (tb + 1) * TB)
                        psq = [ps_acc.tile([128, TB], F32, tag=f"psq{i}",
                                           name=f"psq{i}") for i in range(NQH)]
                        psk = ps_acc.tile([128, TB], F32, tag="psk")
                        psv = ps_acc.tile([128, TB], F32, tag="psv")
                        for kc in range(KC):
                            xt_t = wp.tile([128, TB], F32R, tag="xt_t")
                            nc.sync.dma_start(
                                out=xt_t[:],
                                in_=xT_F[b][:][kc * 128:(kc + 1) * 128, tsl],
                            )
                            for i in range(NQH):
                                nc.tensor.matmul(
                                    psq[i][:],
                                    wq_sb[:, kc * HDQ + i * HD:
                                          kc * HDQ + (i + 1) * HD],
                                    xt_t[:],
                                    start=(kc == 0), stop=(kc == KC - 1),
                                )
                            nc.tensor.matmul(
                                psk[:], wk_sb[:, kc * HD:(kc + 1) * HD], xt_t[:],
                                start=(kc == 0), stop=(kc == KC - 1),
                            )
                            nc.tensor.matmul(
                                psv[:], wv_sb[:, kc * HD:(kc + 1) * HD], xt_t[:],
                                start=(kc == 0), stop=(kc == KC - 1),
                            )

                        cos_t = cos_sb[:, tsl]
                        sin_t = sin_sb[:, tsl]
                        # rope: free each accumulator bank with one ScalarE copy,
                        # then do the math from SBUF
                        for idx in range(NQH + 1):
                            acc = psq[idx] if idx < NQH else psk
                            dest = qT[idx][:] if idx < NQH else kT[:]
                            raw = wp.tile([128, TB], F32R, tag="rope_raw")
                            nc.scalar.copy(raw[:], acc[:])
                            swp = ps_rope.tile([128, TB], F32, tag="swp")
                            nc.tensor.matmul(swp[:], perm_sb[:], raw[:],
                                             start=True, stop=True)
                            t1 = wp.tile([128, TB], F32, tag="rope_t1")
                            nc.vector.tensor_mul(t1[:], raw[:].bitcast(F32), cos_t)
                            t2 = wp.tile([128, TB], F32, tag="rope_t2")
                            nc.vector.tensor_mul(t2[:], swp[:], sin_t)
                            nc.vector.tensor_add(dest[:, tsl], t1[:], t2[:])

                        vt_sb = wp.tile([128, TB], F32, tag="vt_sb")
                        nc.scalar.copy(vt_sb[:], psv[:])
                        for j in range(TB // 128):
                            vp = ps_rope.tile([128, 128], F32, tag="vp")
                            nc.tensor.transpose(
                                vp[:], vt_sb[:, j * 128:(j + 1) * 128], id_sb[:])
                            kt_idx = tb * (TB // 128) + j
                            nc.scalar.copy(
                                v_nat[:, kt_idx * 128:(kt_idx + 1) * 128], vp[:])

                # ---------- attention ----------
                with (
                    tc.tile_pool(name=f"ps_s{b}", bufs=2, space="PSUM") as ps_s,
                    tc.tile_pool(name=f"ps_o{b}", bufs=2, space="PSUM") as ps_o,
                    tc.tile_pool(name=f"ps_sum{b}", bufs=2, space="PSUM") as ps_sum,
                    tc.tile_pool(name=f"wa{b}", bufs=2) as wp,
                ):
                    for h in range(NQH):
                        for qb in range(NQB):
                            q0 = qb * QB
                            kt_max = (q0 + QB) // 128 - 1
                            sT = ps_s.tile([128, QB], F32, tag="sT")
                            oT = ps_o.tile([128, QB], F32, tag="oT")
                            sums = ps_sum.tile([1, QB], F32, tag="sums")
                            pT = wp.tile([128, QB], F32R, tag="pT")
                            for kt in range(kt_max + 1):
                                off = max(0, kt * 128 - q0)
                                qs = slice(q0 + off, q0 + QB)
                                psl = slice(off, QB)
                                nc.tensor.matmul(
                                    sT[:, psl],
                                    kT[:, kt * 128:(kt + 1) * 128],
                                    qT[h][:, qs],
                                    start=True, stop=True,
                                )
                                nc.scalar.activation(
                                    pT[:, psl], sT[:, psl],
                                    mybir.ActivationFunctionType.Exp,
                                    scale=SCALE,
                                )
                                if kt * 128 >= q0:
                                    nc.vector.tensor_mul(
                                        pT[:, off:off + 128],
                                        pT[:, off:off + 128].bitcast(F32),
                                        tri_sb[:],
                                    )
                                nc.tensor.matmul(
                                    oT[:, psl],
                                    v_nat[:, kt * 128:(kt + 1) * 128],
                                    pT[:, psl],
                                    start=(kt == 0), stop=(kt == kt_max),
                                )
                                nc.tensor.matmul(
                                    sums[0:1, psl], ones_sb[:], pT[:, psl],
                                    start=(kt == 0), stop=(kt == kt_max),
                                )
                            sums_sb = wp.tile([1, QB], F32, tag="sums_sb")
                            nc.scalar.copy(sums_sb[:], sums[0:1, :])
                            rec = wp.tile([1, QB], F32, tag="rec")
                            scr = wp.tile([1, QB], F32, tag="scr")
                            nc.vector.reciprocal_approx_accurate(
                                rec[:], sums_sb[:], scr[:])
                            rb = wp.tile([128, QB], F32, tag="rb")
                            nc.gpsimd.partition_broadcast(rb[:], rec[:])
                            oT_sb = wp.tile([128, QB], F32R, tag="oT_sb")
                            nc.vector.tensor_mul(oT_sb[:], oT[:], rb[:])
                            nc.sync.dma_start(
                                out=oT_h[b][:][h * 128:(h + 1) * 128,
                                               q0:q0 + QB],
                                in_=oT_sb[:],
                            )
                nc.gpsimd.collective_compute(
                    "AllGather", mybir.AluOpType.bypass, replica_groups=rg,
                    ins=[oT_h[b][:].opt()], outs=[oT_F[b][:].opt()],
                )

            # ---------- WO projection (column shard) ----------
            bp_cm.__exit__(None, None, None)
            wpool_cm.__exit__(None, None, None)
            with tc.tile_pool(name="wo_p", bufs=1) as wo_p:
                wo_sb = wo_p.tile([128, KC * HDQ], F32R, tag="wo")
                nc.sync.dma_start(
                    out=wo_sb[:].rearrange("p (kc d) -> p kc d", kc=KC),
                    in_=wo.ap().rearrange("(kc p) d -> p kc d", p=128).bitcast(F32R),
                )
                _run_wo(nc, tc, wo_sb, oT_F, y)
    nc.compile()
    return nc


def _run_wo(nc, tc, wo_sb, oT_F, y):
    import concourse.mybir as mybir
    F32 = mybir.dt.float32
    F32R = mybir.dt.float32r
    with (
        tc.tile_pool(name="ps_y", bufs=2, space="PSUM") as ps_y,
        tc.tile_pool(name="w_wo", bufs=2) as wp,
    ):
        for b in range(B):
            for tt in range(S // 128):
                ot_strip = wp.tile([128, KC * 128], F32R, tag="ot_strip")
                nc.sync.dma_start(
                    out=ot_strip[:].rearrange("p (hc t) -> p hc t", hc=KC),
                    in_=oT_F[b][:]
                    .rearrange("(hc p) t -> p hc t", p=128)
                    [:, :, tt * 128:(tt + 1) * 128],
                )
                psy = ps_y.tile([128, HDQ], F32, tag="psy")
                for hc in range(KC):
                    nc.tensor.matmul(
                        psy[:],
                        ot_strip[:, hc * 128:(hc + 1) * 128],
                        wo_sb[:, hc * HDQ:(hc + 1) * HDQ],
                        start=(hc == 0), stop=(hc == KC - 1),
                    )
                y_sb = wp.tile([128, HDQ], F32, tag="y_sb")
                nc.scalar.copy(y_sb[:], psy[:])
                row = b * S + tt * 128
                nc.sync.dma_start(out=y.ap()[row:row + 128, :],
                                  in_=y_sb[:])


def kernel(x, wq, wk, wv, wo, start_pos=0, **_unused):
    from concourse import bass_utils

    x = np.ascontiguousarray(np.asarray(x, dtype=np.float32))
    wq = np.ascontiguousarray(np.asarray(wq, dtype=np.float32))
    wk = np.ascontiguousarray(np.asarray(wk, dtype=np.float32))
    wv = np.ascontiguousarray(np.asarray(wv, dtype=np.float32))
    wo = np.ascontiguousarray(np.asarray(wo, dtype=np.float32))
    assert int(np.asarray(start_pos)) == 0

    x2 = x.reshape(T, DM)
    cosT, sinT, perm, tri, ident, ones = _consts()

    in_maps = []
    for c in range(N_CORES):
        qsl = slice(c * HDQ, (c + 1) * HDQ)
        ksl = slice(c * HD, (c + 1) * HD)
        in_maps.append({
            "xs": np.ascontiguousarray(x2[:, qsl]),
            "wq": np.ascontiguousarray(wq[:, qsl]),
            "wk": np.ascontiguousarray(wk[:, ksl]),
            "wv": np.ascontiguousarray(wv[:, ksl]),
            "wo": np.ascontiguousarray(wo[:, qsl]),
            "cosc": cosT, "sinc": sinT, "permc": perm, "tric": tri,
            "identc": ident, "onesc": ones,
        })

    if "nc" not in _CACHE:
        _CACHE["nc"] = _build()
    nc = _CACHE["nc"]

    res = bass_utils.run_bass_kernel_spmd(
        nc, in_maps, core_ids=list(range(N_CORES)),
        trace=bool(int(os.environ.get("KERNEL_TRACE", "0") or 0)),
    )
    _CACHE["last_result"] = res

    out = np.empty((T, DM), np.float32)
    for c in range(N_CORES):
        out[:, c * HDQ:(c + 1) * HDQ] = res.results[c]["y"]
    return out.reshape(B, S, DM)
